# revision 20
# baseline (speedup 1.0000x reference)
"""MoE layer (16 experts, top-2) + shared SwiGLU MLP on 8 trn2 NeuronCores.

Sharding:
  - MoE experts: expert-parallel, two experts per core. The host computes the
    router (0.2% of the FLOPs), sorts experts by token load and assigns the 8
    heaviest to slot 0 (padded to C0 = max load) and the 8 lightest to slot 1
    (padded to C1 = 9th-largest load), one of each per core. The device runs
    both expert FFNs on the gathered tokens.
  - Shared SwiGLU MLP: hybrid 4-way token x 2-way hidden shard. Core c
    handles token quarter (c % 4) and S-half (c // 4); each core emits a
    partial second-matmul output and the host sums the two S-halves.
  - The host applies the top-2 softmax combine weights, scatter-adds expert
    outputs, and adds the shared-expert output.

Device matmuls default to bf16 operands with fp32 PSUM accumulation
(KMM_DTYPE=fp32|fp32r selects wider operands); outputs return as bf16
(KOUT_DTYPE=fp32 to widen). Every operand is laid out host-side exactly as
its SBUF tile (partition-major), so each DMA is a contiguous-row transfer
and every matmul is lhsT.T @ rhs with no on-device transposes. PSUM->SBUF
output copies rotate across the Vector/Scalar/GpSimd engines so no single
engine's drain rate stalls the matmul pipeline.
"""

import os
import sys
import types

import numpy as np

import concourse.bacc as bacc
import concourse.mybir as mybir
import concourse.tile as tile
from concourse import bass_utils

# bass_utils imports antenv.axon_hooks when tracing is requested; some images
# lack that module. Provide a no-op stub so a stray BASS_TRACE=1 degrades to
# an untraced run instead of crashing (a real hook installed earlier wins).
try:
    import antenv.axon_hooks  # noqa: F401
except ImportError:
    _hooks = types.ModuleType("antenv.axon_hooks")
    _hooks.get_axon_ntff_profile_hook = lambda: None
    _hooks.set_axon_ntff_profile_hook = lambda h: None
    sys.modules["antenv.axon_hooks"] = _hooks
    try:
        import antenv

        antenv.axon_hooks = _hooks
    except ImportError:
        pass

AF = mybir.ActivationFunctionType
FP32 = mybir.dt.float32

B, L, D, H, E, S = 2, 2048, 1024, 512, 16, 2048
T = B * L
TOP_K = 2
NCORES = 8
EPC = E // NCORES   # experts per core
PT = 4              # token-shard ways for the shared expert
PS = 2              # hidden(S)-shard ways for the shared expert
TQ = T // PT        # tokens per core for the shared expert (1024)
SH = S // PS        # hidden units per core for the shared expert (1024)

KD = D // 128       # 8 contraction tiles over D
KH = H // 128       # 4 contraction tiles over H
KSH = SH // 128     # 8 s-tiles per core (its S-half)

MM_DTYPE = os.environ.get("KMM_DTYPE", "bf16")
OUT_DTYPE = os.environ.get("KOUT_DTYPE", "bf16")
_DT = {
    "fp32": mybir.dt.float32,
    "fp32r": mybir.dt.float32r,
    "bf16": mybir.dt.bfloat16,
}

TRACE = False      # set True (or BASS_TRACE=1) to collect an NTFF profile
LAST = None        # BassKernelResults of the most recent run (for test.py)

_PROG_CACHE = {}


def _chunks(total, step=512):
    """Split ``total`` into near-equal chunks <= step (keeps chunks >= 256
    when possible so float32r matmuls stay at the 1 cyc/row rate)."""
    n = max(1, -(-total // step))
    base = total // n
    rem = total - base * n
    out, off = [], 0
    for i in range(n):
        w = base + (1 if i < rem else 0)
        out.append((off, w))
        off += w
    return out


def _pmajor(a, cols):
    """[K, M] k-major matrix -> [128, (K/128)*M] partition-major image whose
    columns are the K-tiles side by side; ``cols`` = M per tile."""
    K, M = a.shape
    assert M == cols
    return np.ascontiguousarray(
        a.reshape(K // 128, 128, M).transpose(1, 0, 2).reshape(128, -1)
    )


def build_program(C0, C1, mmdt_key, outdt_key):
    mmdt = _DT[mmdt_key]
    outdt = FP32 if outdt_key in ("fp32", "fp32r") else _DT[outdt_key]
    nc = bacc.Bacc(
        "TRN2", target_bir_lowering=False, debug=False, enable_asserts=False
    )

    # xq grid: [half*2 + tc, 128, 2048] — each entry is one token chunk (tc)
    # of four j-tiles packed contiguously, so every xq DMA moves 4KB-per-row
    # descriptors (1KB descriptors throttle the early descriptor-rate-bound
    # window to ~2/3 bandwidth).
    xq = nc.dram_tensor("xq", [4, 128, 2048], mmdt, kind="ExternalInput").ap()
    # per s-tile: 8 sfc1 k-tiles then 8 sfc2 k-tiles, side by side
    sfc12 = nc.dram_tensor("sfc12", [KSH, 128, 2 * KD * 128], mmdt, kind="ExternalInput").ap()
    # per d-tile: the core's 8 sfc3 s-tiles
    sfc3h = nc.dram_tensor("sfc3h", [KD, 128, KSH * 128], mmdt, kind="ExternalInput").ap()
    xg0 = nc.dram_tensor("xg0", [128, KD * C0], mmdt, kind="ExternalInput").ap()
    xg1 = nc.dram_tensor("xg1", [128, KD * C1], mmdt, kind="ExternalInput").ap()
    w1b = nc.dram_tensor("w1b", [EPC, 128, KH * KD * 128], mmdt, kind="ExternalInput").ap()
    w2b = nc.dram_tensor("w2b", [EPC, 128, KD * KH * 128], mmdt, kind="ExternalInput").ap()
    pshout = nc.dram_tensor("pshout", [KD, 128, TQ], outdt, kind="ExternalOutput").ap()
    yout0 = nc.dram_tensor("yout0", [KD, 128, C0], outdt, kind="ExternalOutput").ap()
    yout1 = nc.dram_tensor("yout1", [KD, 128, C1], outdt, kind="ExternalOutput").ap()

    tch = _chunks(TQ)   # token chunks for the shared expert (2 x 512)
    cchs = [_chunks(C0), _chunks(C1)]
    xgs = [xg0, xg1]
    youts = [yout0, yout1]

    with tile.TileContext(nc) as tc:
        with (
            tc.tile_pool(name="bigp", bufs=2) as bigp,
            tc.tile_pool(name="gp", bufs=1) as gp,
            tc.tile_pool(name="w12p", bufs=KSH) as w12p,
            tc.tile_pool(name="w3p", bufs=KD) as w3p,
            tc.tile_pool(name="w1p", bufs=2) as w1p,
            tc.tile_pool(name="w2p", bufs=2) as w2p,
            tc.tile_pool(name="xep", bufs=2) as xep,
            tc.tile_pool(name="hp", bufs=2) as hp,
            tc.tile_pool(name="sap", bufs=2) as sap,
            tc.tile_pool(name="obp", bufs=4) as obp,
            tc.tile_pool(name="ps", bufs=8, space="PSUM") as ps,
        ):
            # PSUM->SBUF output copies alternate engines per d-tile so the
            # drain never gates PSUM-bank reuse during the w2 / stage-B
            # phases (GpSimd cannot read PSUM on TRN2), and each engine
            # issues its own output-DMA trigger. Keeping output triggers off
            # the sync queue matters: sync issues every input load in order,
            # and an output trigger parked there waiting on its copy would
            # stall all later input loads behind it.
            _cp = [
                nc.vector.tensor_copy,
                lambda o, i: nc.scalar.activation(o, i, AF.Copy),
            ]
            # Vector can't trigger DMAs itself; the idle GpSimd queue issues
            # the triggers for vector-copied tiles instead.
            _dma = [nc.gpsimd.dma_start, nc.scalar.dma_start]

            # PE warmup: junk matmuls on a memset tile keep the tensor
            # engine busy from engine-start (~6us) until the first input
            # DMAs land (~12.4us), walking the DVFS p-state ramp
            # (0.65 -> 1.2 -> 2.4 GHz, ~3us of busy time) on throwaway work
            # so the real window opens at full clock.
            wu = sap.tile([128, 512], mmdt, tag="wu", name="warmup")
            nc.gpsimd.memset(wu[:], 0.0)
            pw = ps.tile([128, 512], FP32, tag="ps")
            for _ in range(14):
                nc.tensor.matmul(pw[:], wu[:, :128], wu[:], start=True, stop=True)

            # All 8 w12 s-tiles stay resident; DMAs are issued in the order
            # the matmul stream consumes them (sfc1-half of s-tile 0 and the
            # first token chunk first) so the window opens as early as the
            # HBM stream allows.
            w12 = [
                w12p.tile([128, 2 * KD * 128], mmdt, tag="w12", name=f"w12_{st}")
                for st in range(KSH)
            ]
            nc.sync.dma_start(out=w12[0][:, : KD * 128], in_=sfc12[0, :, : KD * 128])
            xq_a = bigp.tile([128, 4 * TQ], mmdt, tag="big", name="xq_a")
            xq_b = bigp.tile([128, 4 * TQ], mmdt, tag="big", name="xq_b")
            xq_half = [xq_a, xq_b]

            def xq_sl(j, off, w):
                # SBUF cols: tc*2048 + (j%4)*512 + (off within chunk); stage-A
                # chunks never straddle the 512 boundary (tch is 512-aligned)
                tci, o = divmod(off, 512)
                base = tci * 2048 + (j % 4) * 512 + o
                return xq_half[j // 4][:, base:base + w]

            for h in range(2):
                nc.sync.dma_start(
                    out=xq_half[h][:, :2048], in_=xq[2 * h]
                )
            nc.sync.dma_start(out=w12[0][:, KD * 128:], in_=sfc12[0, :, KD * 128:])
            nc.sync.dma_start(out=w12[1][:], in_=sfc12[1])
            nc.sync.dma_start(out=w12[2][:], in_=sfc12[2])
            for h in range(2):
                nc.sync.dma_start(
                    out=xq_half[h][:, 2048:], in_=xq[2 * h + 1]
                )
            for st in range(3, KSH):
                nc.sync.dma_start(out=w12[st][:], in_=sfc12[st])

            # g[s, t] = silu(x@sfc1.T) * (x@sfc2.T) for this core's S-half.
            # Chunk-outer: the whole first pass needs only token chunk 0, so
            # the ramp-in isn't waiting on the full xq stream.
            g_t = gp.tile([128, KSH * TQ], mmdt)

            def stage_a_pass(off, w):
                for st in range(KSH):
                    pa = ps.tile([128, 512], FP32, tag="ps")
                    for j in range(KD):
                        nc.tensor.matmul(
                            pa[:, :w], w12[st][:, j * 128:(j + 1) * 128],
                            xq_sl(j, off, w),
                            start=(j == 0), stop=(j == KD - 1),
                        )
                    sa = sap.tile([128, 512], FP32, tag="sa")
                    nc.scalar.activation(sa[:, :w], pa[:, :w], AF.Silu)
                    pb = ps.tile([128, 512], FP32, tag="ps")
                    for j in range(KD):
                        nc.tensor.matmul(
                            pb[:, :w], w12[st][:, (KD + j) * 128:(KD + j + 1) * 128],
                            xq_sl(j, off, w),
                            start=(j == 0), stop=(j == KD - 1),
                        )
                    nc.vector.tensor_mul(
                        g_t[:, st * TQ + off:st * TQ + off + w], sa[:, :w], pb[:, :w]
                    )

            # owned experts: y_e = silu(x_e @ w1.T) @ w2.T on gathered
            # tokens. Slot 0's weights prefetch into dedicated half-tiles
            # during stage A; slot 1's weights recycle the xq slots (free
            # at the end of stage A) and stream during stage B.
            full = KH * KD * 128
            half = full // 2

            def emit_expert(s):
                C = (C0, C1)[s]
                cch = cchs[s]
                xe_t = xep.tile([128, KD * C], mmdt, tag="xe", name=f"xe{s}")
                nc.sync.dma_start(out=xe_t[:], in_=xgs[s])
                h_t = hp.tile([128, KH * C], mmdt, tag="h", name=f"h{s}")
                if s == 0:
                    w1t = [None, None]
                    for hf in range(2):
                        w1t[hf] = w1p.tile([128, half], mmdt, tag="w1", name=f"w1t{s}_{hf}")
                        nc.sync.dma_start(
                            out=w1t[hf][:], in_=w1b[s, :, hf * half:(hf + 1) * half]
                        )
                    w1sl = lambda ht, j: w1t[ht // 2][:, ((ht % 2) * KD + j) * 128:((ht % 2) * KD + j + 1) * 128]
                else:
                    w1f = bigp.tile([128, full], mmdt, tag="big", name="w1t_e1")
                    nc.sync.dma_start(out=w1f[:], in_=w1b[s])
                    w1sl = lambda ht, j: w1f[:, (ht * KD + j) * 128:(ht * KD + j + 1) * 128]
                for ht in range(KH):
                    for off, w in cch:
                        ph = ps.tile([128, 512], FP32, tag="ps")
                        for j in range(KD):
                            nc.tensor.matmul(
                                ph[:, :w], w1sl(ht, j),
                                xe_t[:, j * C + off:j * C + off + w],
                                start=(j == 0), stop=(j == KD - 1),
                            )
                        nc.scalar.activation(
                            h_t[:, ht * C + off:ht * C + off + w], ph[:, :w], AF.Silu
                        )
                if s == 0:
                    w2t = [None, None]
                    for hf in range(2):
                        w2t[hf] = w2p.tile([128, half], mmdt, tag="w2", name=f"w2t{s}_{hf}")
                        nc.sync.dma_start(
                            out=w2t[hf][:], in_=w2b[s, :, hf * half:(hf + 1) * half]
                        )
                    w2sl = lambda dt, j: w2t[dt // 4][:, ((dt % 4) * KH + j) * 128:((dt % 4) * KH + j + 1) * 128]
                else:
                    w2f = bigp.tile([128, full], mmdt, tag="big", name="w2t_e1")
                    nc.sync.dma_start(out=w2f[:], in_=w2b[s])
                    w2sl = lambda dt, j: w2f[:, (dt * KH + j) * 128:(dt * KH + j + 1) * 128]
                for dt in range(KD):
                    eng = dt % 2
                    # the last d-tiles of the program drain per chunk so
                    # their output DMAs overlap the final compute+copy
                    tail_split = s == 1 and dt >= KD - 2 and len(cch) > 1
                    yo = obp.tile([128, C], outdt, tag="ob")
                    for off, w in cch:
                        py = ps.tile([128, 512], FP32, tag="ps")
                        for j in range(KH):
                            nc.tensor.matmul(
                                py[:, :w], w2sl(dt, j),
                                h_t[:, j * C + off:j * C + off + w],
                                start=(j == 0), stop=(j == KH - 1),
                            )
                        _cp[eng](yo[:, off:off + w], py[:, :w])
                        if tail_split:
                            _dma[eng](
                                out=youts[s][dt, :, off:off + w],
                                in_=yo[:, off:off + w],
                            )
                    if not tail_split:
                        _dma[eng](out=youts[s][dt], in_=yo[:])

            def emit_stage_b():
                # partial shared second matmul over this core's S-half:
                # pshout[d, t] = sum_{s in half} sfc3[d, s] * g[s, t]
                for dt in range(KD):
                    w3t = w3p.tile([128, KSH * 128], mmdt, tag="w3")
                    nc.sync.dma_start(out=w3t[:], in_=sfc3h[dt])
                    eng = dt % 2
                    po = obp.tile([128, TQ], outdt, tag="ob")
                    for off, w in tch:
                        pc = ps.tile([128, 512], FP32, tag="ps")
                        for sj in range(KSH):
                            nc.tensor.matmul(
                                pc[:, :w], w3t[:, sj * 128:(sj + 1) * 128],
                                g_t[:, sj * TQ + off:sj * TQ + off + w],
                                start=(sj == 0), stop=(sj == KSH - 1),
                            )
                        _cp[eng](po[:, off:off + w], pc[:, :w])
                    _dma[eng](out=pshout[dt], in_=po[:])

            for off, w in tch:
                stage_a_pass(off, w)
            emit_expert(0)
            emit_stage_b()
            emit_expert(1)

    nc.compile()
    return nc


def kernel(**inputs):
    global LAST
    x = np.ascontiguousarray(np.asarray(inputs["x"], dtype=np.float32))
    gate_w = np.asarray(inputs["gate_w"], dtype=np.float32)
    w1 = np.asarray(inputs["w1"], dtype=np.float32)
    w2 = np.asarray(inputs["w2"], dtype=np.float32)
    sfc1 = np.asarray(inputs["sfc1"], dtype=np.float32)
    sfc2 = np.asarray(inputs["sfc2"], dtype=np.float32)
    sfc3 = np.asarray(inputs["sfc3"], dtype=np.float32)

    xf = x.reshape(T, D)

    # router on host (tiny): top-2 of 16 logits, softmax over the pair
    logits = xf @ gate_w.T
    idx = np.argpartition(-logits, TOP_K, axis=1)[:, :TOP_K]
    lg = np.take_along_axis(logits, idx, axis=1)
    m = lg.max(axis=1, keepdims=True)
    p = np.exp(lg - m)
    wk = (p / p.sum(axis=1, keepdims=True)).astype(np.float32)

    toks, wts = [], []
    for e in range(E):
        sel = idx == e
        rows = np.nonzero(sel.any(axis=1))[0]
        toks.append(rows)
        wts.append(wk[sel])

    # slot packing: the 8 heaviest experts pad to C0 = max load, the 8
    # lightest pad to C1 = 9th-largest load
    loads = np.array([len(r) for r in toks])
    order = np.argsort(-loads, kind="stable")
    slots = [list(order[:NCORES]), list(order[NCORES:])]
    rnd = lambda n: max(((int(n) + 7) // 8) * 8, 256)
    C0 = rnd(loads[slots[0]].max())
    C1 = rnd(loads[slots[1]].max())

    key = (C0, C1, MM_DTYPE, OUT_DTYPE)
    if key not in _PROG_CACHE:
        _PROG_CACHE[key] = build_program(C0, C1, MM_DTYPE, OUT_DTYPE)
    nc = _PROG_CACHE[key]
    np_mm = mybir.dt.np(_DT[MM_DTYPE])

    sfc1T = np.ascontiguousarray(sfc1.T)   # [D, S]
    sfc2T = np.ascontiguousarray(sfc2.T)
    sfc3T = np.ascontiguousarray(sfc3.T)   # [S, D]

    # sfc12 per S-half: [KSH, 128, 2*KD*128]
    sfc12_h, sfc3_h = [], []
    for sh in range(PS):
        blk = np.empty((KSH, 128, 2 * KD * 128), np.float32)
        for st in range(KSH):
            s0 = (sh * KSH + st) * 128
            a = sfc1T[:, s0:s0 + 128]    # [D, 128]
            b = sfc2T[:, s0:s0 + 128]
            blk[st, :, : KD * 128] = _pmajor(a, 128)
            blk[st, :, KD * 128:] = _pmajor(b, 128)
        sfc12_h.append(blk.astype(np_mm))
        blk3 = np.empty((KD, 128, KSH * 128), np.float32)
        s0 = sh * SH
        for dt in range(KD):
            # [SH, 128] slice of sfc3T -> partition-major over its s-tiles
            blk3[dt] = _pmajor(
                np.ascontiguousarray(sfc3T[s0:s0 + SH, dt * 128:(dt + 1) * 128]), 128
            )
        sfc3_h.append(blk3.astype(np_mm))

    in_maps = []
    for c in range(NCORES):
        q, sh = c % PT, c // PT
        xqm = _pmajor(
            np.ascontiguousarray(xf[q * TQ:(q + 1) * TQ].T), TQ
        ).astype(np_mm)
        # [128, KD*TQ] -> grid [h*2 + tc, 128, (j%4)*512 + c]: one packed
        # 4KB-per-row transfer per (xq half, token chunk)
        xqm = np.ascontiguousarray(
            xqm.reshape(128, 2, 4, 2, 512).transpose(1, 3, 0, 2, 4).reshape(4, 128, 2048)
        )
        in_map = {
            "xq": xqm,
            "sfc12": sfc12_h[sh],
            "sfc3h": sfc3_h[sh],
        }
        w1_c, w2_c = [], []
        for s, C in ((0, C0), (1, C1)):
            e = slots[s][c]
            rows = toks[e]
            xe = np.zeros((C, D), np.float32)
            xe[: len(rows)] = xf[rows]
            in_map[f"xg{s}"] = _pmajor(np.ascontiguousarray(xe.T), C).astype(np_mm)
            # w1 tiles keyed (ht, j): col block (ht*KD + j) is k-tile j of
            # w1[e].T's h-tile ht
            w1T = np.ascontiguousarray(w1[e].T)   # [D, H]
            w1m = np.empty((128, KH * KD * 128), np.float32)
            for ht in range(KH):
                w1m[:, ht * KD * 128:(ht + 1) * KD * 128] = _pmajor(
                    np.ascontiguousarray(w1T[:, ht * 128:(ht + 1) * 128]), 128
                )
            w1_c.append(w1m)
            # w2 tiles keyed (dt, hj)
            w2T = np.ascontiguousarray(w2[e].T)   # [H, D]
            w2m = np.empty((128, KD * KH * 128), np.float32)
            for dt in range(KD):
                w2m[:, dt * KH * 128:(dt + 1) * KH * 128] = _pmajor(
                    np.ascontiguousarray(w2T[:, dt * 128:(dt + 1) * 128]), 128
                )
            w2_c.append(w2m)
        in_map["w1b"] = np.stack(w1_c).astype(np_mm)
        in_map["w2b"] = np.stack(w2_c).astype(np_mm)
        in_maps.append(in_map)

    trace = TRACE or os.environ.get("BASS_TRACE") == "1"
    res = bass_utils.run_bass_kernel_spmd(
        nc, in_maps, core_ids=list(range(NCORES)), trace=trace
    )
    LAST = res
    results = res.results

    out = np.empty((T, D), np.float32)
    for q in range(PT):
        acc = np.asarray(results[q]["pshout"]).astype(np.float32).reshape(D, TQ)
        acc = acc + np.asarray(results[PT + q]["pshout"]).astype(np.float32).reshape(D, TQ)
        out[q * TQ:(q + 1) * TQ] = acc.T
    for s, C in ((0, C0), (1, C1)):
        for c in range(NCORES):
            e = slots[s][c]
            load = len(toks[e])
            yT = np.asarray(results[c][f"yout{s}"]).astype(np.float32).reshape(D, C)
            out[toks[e]] += wts[e][:, None] * yT[:, :load].T
    return out.reshape(B, L, D)


# revision 21
# speedup vs baseline: 1.1808x; 1.1808x over previous
"""MoE layer (16 experts, top-2) + shared SwiGLU MLP on 8 trn2 NeuronCores.

Sharding:
  - MoE experts: expert-parallel, two experts per core. The host computes the
    router (0.2% of the FLOPs), sorts experts by token load and assigns the 8
    heaviest to slot 0 (padded to C0 = max load) and the 8 lightest to slot 1
    (padded to C1 = 9th-largest load), one of each per core. The device runs
    both expert FFNs on the gathered tokens.
  - Shared SwiGLU MLP: hybrid 4-way token x 2-way hidden shard. Core c
    handles token quarter (c % 4) and S-half (c // 4); each core emits a
    partial second-matmul output and the host sums the two S-halves.
  - The host applies the top-2 softmax combine weights, scatter-adds expert
    outputs, and adds the shared-expert output.

Device matmuls default to bf16 operands with fp32 PSUM accumulation
(KMM_DTYPE=fp32|fp32r selects wider operands); outputs return as bf16
(KOUT_DTYPE=fp32 to widen). Every operand is laid out host-side exactly as
its SBUF tile (partition-major), so each DMA is a contiguous-row transfer
and every matmul is lhsT.T @ rhs with no on-device transposes. PSUM->SBUF
output copies rotate across the Vector/Scalar/GpSimd engines so no single
engine's drain rate stalls the matmul pipeline.
"""

import os
import sys
import types

import numpy as np

import concourse.bacc as bacc
import concourse.mybir as mybir
import concourse.tile as tile
from concourse import bass_utils

# bass_utils imports antenv.axon_hooks when tracing is requested; some images
# lack that module. Provide a no-op stub so a stray BASS_TRACE=1 degrades to
# an untraced run instead of crashing (a real hook installed earlier wins).
try:
    import antenv.axon_hooks  # noqa: F401
except ImportError:
    _hooks = types.ModuleType("antenv.axon_hooks")
    _hooks.get_axon_ntff_profile_hook = lambda: None
    _hooks.set_axon_ntff_profile_hook = lambda h: None
    sys.modules["antenv.axon_hooks"] = _hooks
    try:
        import antenv

        antenv.axon_hooks = _hooks
    except ImportError:
        pass

AF = mybir.ActivationFunctionType
FP32 = mybir.dt.float32

B, L, D, H, E, S = 2, 2048, 1024, 512, 16, 2048
T = B * L
TOP_K = 2
NCORES = 8
EPC = E // NCORES   # experts per core
PT = 4              # token-shard ways for the shared expert
PS = 2              # hidden(S)-shard ways for the shared expert
TQ = T // PT        # tokens per core for the shared expert (1024)
SH = S // PS        # hidden units per core for the shared expert (1024)

KD = D // 128       # 8 contraction tiles over D
KH = H // 128       # 4 contraction tiles over H
KSH = SH // 128     # 8 s-tiles per core (its S-half)

MM_DTYPE = os.environ.get("KMM_DTYPE", "bf16")
OUT_DTYPE = os.environ.get("KOUT_DTYPE", "bf16")
_DT = {
    "fp32": mybir.dt.float32,
    "fp32r": mybir.dt.float32r,
    "bf16": mybir.dt.bfloat16,
}

TRACE = False      # set True (or BASS_TRACE=1) to collect an NTFF profile
LAST = None        # BassKernelResults of the most recent run (for test.py)

_PROG_CACHE = {}


def _chunks(total, step=512):
    """Split ``total`` into near-equal chunks <= step (keeps chunks >= 256
    when possible so float32r matmuls stay at the 1 cyc/row rate)."""
    n = max(1, -(-total // step))
    base = total // n
    rem = total - base * n
    out, off = [], 0
    for i in range(n):
        w = base + (1 if i < rem else 0)
        out.append((off, w))
        off += w
    return out


def _pmajor(a, cols):
    """[K, M] k-major matrix -> [128, (K/128)*M] partition-major image whose
    columns are the K-tiles side by side; ``cols`` = M per tile."""
    K, M = a.shape
    assert M == cols
    return np.ascontiguousarray(
        a.reshape(K // 128, 128, M).transpose(1, 0, 2).reshape(128, -1)
    )


def build_program(C0, C1, mmdt_key, outdt_key):
    mmdt = _DT[mmdt_key]
    outdt = FP32 if outdt_key in ("fp32", "fp32r") else _DT[outdt_key]
    nc = bacc.Bacc(
        "TRN2", target_bir_lowering=False, debug=False, enable_asserts=False
    )

    # xq grid: [half*2 + tc, 128, 2048] — each entry is one token chunk (tc)
    # of four j-tiles packed contiguously, so every xq DMA moves 4KB-per-row
    # descriptors (1KB descriptors throttle the early descriptor-rate-bound
    # window to ~2/3 bandwidth).
    xq = nc.dram_tensor("xq", [4, 128, 2048], mmdt, kind="ExternalInput").ap()
    # per s-tile: 8 sfc1 k-tiles then 8 sfc2 k-tiles, side by side
    sfc12 = nc.dram_tensor("sfc12", [KSH, 128, 2 * KD * 128], mmdt, kind="ExternalInput").ap()
    # per d-tile: the core's 8 sfc3 s-tiles
    sfc3h = nc.dram_tensor("sfc3h", [KD, 128, KSH * 128], mmdt, kind="ExternalInput").ap()
    xg0 = nc.dram_tensor("xg0", [128, KD * C0], mmdt, kind="ExternalInput").ap()
    xg1 = nc.dram_tensor("xg1", [128, KD * C1], mmdt, kind="ExternalInput").ap()
    w1b = nc.dram_tensor("w1b", [EPC, 128, KH * KD * 128], mmdt, kind="ExternalInput").ap()
    w2b = nc.dram_tensor("w2b", [EPC, 128, KD * KH * 128], mmdt, kind="ExternalInput").ap()
    pshout = nc.dram_tensor("pshout", [KD, 128, TQ], outdt, kind="ExternalOutput").ap()
    yout0 = nc.dram_tensor("yout0", [KD, 128, C0], outdt, kind="ExternalOutput").ap()
    yout1 = nc.dram_tensor("yout1", [KD, 128, C1], outdt, kind="ExternalOutput").ap()

    tch = _chunks(TQ)   # token chunks for the shared expert (2 x 512)
    cchs = [_chunks(C0), _chunks(C1)]
    xgs = [xg0, xg1]
    youts = [yout0, yout1]

    with tile.TileContext(nc) as tc:
        with (
            tc.tile_pool(name="bigp", bufs=2) as bigp,
            tc.tile_pool(name="gp", bufs=1) as gp,
            tc.tile_pool(name="w12p", bufs=KSH) as w12p,
            tc.tile_pool(name="w3p", bufs=KD) as w3p,
            tc.tile_pool(name="w1p", bufs=2) as w1p,
            tc.tile_pool(name="w2p", bufs=2) as w2p,
            tc.tile_pool(name="xep", bufs=2) as xep,
            tc.tile_pool(name="hp", bufs=2) as hp,
            tc.tile_pool(name="sap", bufs=2) as sap,
            tc.tile_pool(name="obp", bufs=4) as obp,
            tc.tile_pool(name="ps", bufs=8, space="PSUM") as ps,
        ):
            # PSUM->SBUF output copies alternate engines per d-tile so the
            # drain never gates PSUM-bank reuse during the w2 / stage-B
            # phases (GpSimd cannot read PSUM on TRN2), and each engine
            # issues its own output-DMA trigger. Keeping output triggers off
            # the sync queue matters: sync issues every input load in order,
            # and an output trigger parked there waiting on its copy would
            # stall all later input loads behind it.
            _cp = [
                nc.vector.tensor_copy,
                lambda o, i: nc.scalar.activation(o, i, AF.Copy),
            ]
            # Vector can't trigger DMAs itself; the idle GpSimd queue issues
            # the triggers for vector-copied tiles instead.
            _dma = [nc.gpsimd.dma_start, nc.scalar.dma_start]

            # PE warmup: junk matmuls on a memset tile keep the tensor
            # engine busy from engine-start (~6us) until the first input
            # DMAs land (~12.4us), walking the DVFS p-state ramp
            # (0.65 -> 1.2 -> 2.4 GHz, ~3us of busy time) on throwaway work
            # so the real window opens at full clock.
            wu = sap.tile([128, 512], mmdt, tag="wu", name="warmup")
            nc.gpsimd.memset(wu[:], 0.0)
            pw = ps.tile([128, 512], FP32, tag="ps")
            for _ in range(14):
                nc.tensor.matmul(pw[:], wu[:, :128], wu[:], start=True, stop=True)

            # All 8 w12 s-tiles stay resident; DMAs are issued in the order
            # the matmul stream consumes them (sfc1-half of s-tile 0 and the
            # first token chunk first) so the window opens as early as the
            # HBM stream allows.
            w12 = [
                w12p.tile([128, 2 * KD * 128], mmdt, tag="w12", name=f"w12_{st}")
                for st in range(KSH)
            ]
            nc.sync.dma_start(out=w12[0][:, : KD * 128], in_=sfc12[0, :, : KD * 128])
            xq_a = bigp.tile([128, 4 * TQ], mmdt, tag="big", name="xq_a")
            xq_b = bigp.tile([128, 4 * TQ], mmdt, tag="big", name="xq_b")
            xq_half = [xq_a, xq_b]

            def xq_sl(j, off, w):
                # SBUF cols: tc*2048 + (j%4)*512 + (off within chunk); stage-A
                # chunks never straddle the 512 boundary (tch is 512-aligned)
                tci, o = divmod(off, 512)
                base = tci * 2048 + (j % 4) * 512 + o
                return xq_half[j // 4][:, base:base + w]

            for h in range(2):
                nc.sync.dma_start(
                    out=xq_half[h][:, :2048], in_=xq[2 * h]
                )
            nc.sync.dma_start(out=w12[0][:, KD * 128:], in_=sfc12[0, :, KD * 128:])
            nc.sync.dma_start(out=w12[1][:], in_=sfc12[1])
            nc.sync.dma_start(out=w12[2][:], in_=sfc12[2])
            for h in range(2):
                nc.sync.dma_start(
                    out=xq_half[h][:, 2048:], in_=xq[2 * h + 1]
                )
            for st in range(3, KSH):
                nc.sync.dma_start(out=w12[st][:], in_=sfc12[st])

            # g[s, t] = silu(x@sfc1.T) * (x@sfc2.T) for this core's S-half.
            # Chunk-outer: the whole first pass needs only token chunk 0, so
            # the ramp-in isn't waiting on the full xq stream.
            g_t = gp.tile([128, KSH * TQ], mmdt)

            def stage_a_pass(off, w):
                for st in range(KSH):
                    pa = ps.tile([128, 512], FP32, tag="ps")
                    for j in range(KD):
                        nc.tensor.matmul(
                            pa[:, :w], w12[st][:, j * 128:(j + 1) * 128],
                            xq_sl(j, off, w),
                            start=(j == 0), stop=(j == KD - 1),
                        )
                    sa = sap.tile([128, 512], FP32, tag="sa")
                    nc.scalar.activation(sa[:, :w], pa[:, :w], AF.Silu)
                    pb = ps.tile([128, 512], FP32, tag="ps")
                    for j in range(KD):
                        nc.tensor.matmul(
                            pb[:, :w], w12[st][:, (KD + j) * 128:(KD + j + 1) * 128],
                            xq_sl(j, off, w),
                            start=(j == 0), stop=(j == KD - 1),
                        )
                    nc.vector.tensor_mul(
                        g_t[:, st * TQ + off:st * TQ + off + w], sa[:, :w], pb[:, :w]
                    )

            # owned experts: y_e = silu(x_e @ w1.T) @ w2.T on gathered
            # tokens. Slot 0's weights prefetch into dedicated half-tiles
            # during stage A; slot 1's weights recycle the xq slots (free
            # at the end of stage A) and stream during stage B.
            full = KH * KD * 128
            half = full // 2

            def emit_expert(s):
                C = (C0, C1)[s]
                cch = cchs[s]
                xe_t = xep.tile([128, KD * C], mmdt, tag="xe", name=f"xe{s}")
                nc.sync.dma_start(out=xe_t[:], in_=xgs[s])
                h_t = hp.tile([128, KH * C], mmdt, tag="h", name=f"h{s}")
                if s == 0:
                    w1t = [None, None]
                    for hf in range(2):
                        w1t[hf] = w1p.tile([128, half], mmdt, tag="w1", name=f"w1t{s}_{hf}")
                        nc.sync.dma_start(
                            out=w1t[hf][:], in_=w1b[s, :, hf * half:(hf + 1) * half]
                        )
                    w1sl = lambda ht, j: w1t[ht // 2][:, ((ht % 2) * KD + j) * 128:((ht % 2) * KD + j + 1) * 128]
                else:
                    w1f = bigp.tile([128, full], mmdt, tag="big", name="w1t_e1")
                    nc.sync.dma_start(out=w1f[:], in_=w1b[s])
                    w1sl = lambda ht, j: w1f[:, (ht * KD + j) * 128:(ht * KD + j + 1) * 128]
                for ht in range(KH):
                    for off, w in cch:
                        ph = ps.tile([128, 512], FP32, tag="ps")
                        for j in range(KD):
                            nc.tensor.matmul(
                                ph[:, :w], w1sl(ht, j),
                                xe_t[:, j * C + off:j * C + off + w],
                                start=(j == 0), stop=(j == KD - 1),
                            )
                        nc.scalar.activation(
                            h_t[:, ht * C + off:ht * C + off + w], ph[:, :w], AF.Silu
                        )
                if s == 0:
                    w2t = [None, None]
                    for hf in range(2):
                        w2t[hf] = w2p.tile([128, half], mmdt, tag="w2", name=f"w2t{s}_{hf}")
                        nc.sync.dma_start(
                            out=w2t[hf][:], in_=w2b[s, :, hf * half:(hf + 1) * half]
                        )
                    w2sl = lambda dt, j: w2t[dt // 4][:, ((dt % 4) * KH + j) * 128:((dt % 4) * KH + j + 1) * 128]
                else:
                    w2f = bigp.tile([128, full], mmdt, tag="big", name="w2t_e1")
                    nc.sync.dma_start(out=w2f[:], in_=w2b[s])
                    w2sl = lambda dt, j: w2f[:, (dt * KH + j) * 128:(dt * KH + j + 1) * 128]
                for dt in range(KD):
                    eng = dt % 2
                    # the last d-tiles of the program drain per chunk so
                    # their output DMAs overlap the final compute+copy, and
                    # always trigger via scalar — a gpsimd-issued trigger
                    # costs ~650ns of software descriptor generation, which
                    # lands squarely in the drain tail
                    tail = s == 1 and dt >= KD - 2
                    trig = _dma[1] if tail else _dma[eng]
                    tail_split = tail and len(cch) > 1
                    yo = obp.tile([128, C], outdt, tag="ob")
                    for off, w in cch:
                        py = ps.tile([128, 512], FP32, tag="ps")
                        for j in range(KH):
                            nc.tensor.matmul(
                                py[:, :w], w2sl(dt, j),
                                h_t[:, j * C + off:j * C + off + w],
                                start=(j == 0), stop=(j == KH - 1),
                            )
                        _cp[eng](yo[:, off:off + w], py[:, :w])
                        if tail_split:
                            trig(
                                out=youts[s][dt, :, off:off + w],
                                in_=yo[:, off:off + w],
                            )
                    if not tail_split:
                        trig(out=youts[s][dt], in_=yo[:])

            def emit_stage_b():
                # partial shared second matmul over this core's S-half:
                # pshout[d, t] = sum_{s in half} sfc3[d, s] * g[s, t]
                for dt in range(KD):
                    w3t = w3p.tile([128, KSH * 128], mmdt, tag="w3")
                    nc.sync.dma_start(out=w3t[:], in_=sfc3h[dt])
                    eng = dt % 2
                    po = obp.tile([128, TQ], outdt, tag="ob")
                    for off, w in tch:
                        pc = ps.tile([128, 512], FP32, tag="ps")
                        for sj in range(KSH):
                            nc.tensor.matmul(
                                pc[:, :w], w3t[:, sj * 128:(sj + 1) * 128],
                                g_t[:, sj * TQ + off:sj * TQ + off + w],
                                start=(sj == 0), stop=(sj == KSH - 1),
                            )
                        _cp[eng](po[:, off:off + w], pc[:, :w])
                    _dma[eng](out=pshout[dt], in_=po[:])

            for off, w in tch:
                stage_a_pass(off, w)
            emit_expert(0)
            emit_stage_b()
            emit_expert(1)

    nc.compile()
    return nc


def kernel(**inputs):
    global LAST
    x = np.ascontiguousarray(np.asarray(inputs["x"], dtype=np.float32))
    gate_w = np.asarray(inputs["gate_w"], dtype=np.float32)
    w1 = np.asarray(inputs["w1"], dtype=np.float32)
    w2 = np.asarray(inputs["w2"], dtype=np.float32)
    sfc1 = np.asarray(inputs["sfc1"], dtype=np.float32)
    sfc2 = np.asarray(inputs["sfc2"], dtype=np.float32)
    sfc3 = np.asarray(inputs["sfc3"], dtype=np.float32)

    xf = x.reshape(T, D)

    # router on host (tiny): top-2 of 16 logits, softmax over the pair
    logits = xf @ gate_w.T
    idx = np.argpartition(-logits, TOP_K, axis=1)[:, :TOP_K]
    lg = np.take_along_axis(logits, idx, axis=1)
    m = lg.max(axis=1, keepdims=True)
    p = np.exp(lg - m)
    wk = (p / p.sum(axis=1, keepdims=True)).astype(np.float32)

    toks, wts = [], []
    for e in range(E):
        sel = idx == e
        rows = np.nonzero(sel.any(axis=1))[0]
        toks.append(rows)
        wts.append(wk[sel])

    # slot packing: the 8 heaviest experts pad to C0 = max load, the 8
    # lightest pad to C1 = 9th-largest load
    loads = np.array([len(r) for r in toks])
    order = np.argsort(-loads, kind="stable")
    slots = [list(order[:NCORES]), list(order[NCORES:])]
    rnd = lambda n: max(((int(n) + 7) // 8) * 8, 256)
    C0 = rnd(loads[slots[0]].max())
    C1 = rnd(loads[slots[1]].max())

    key = (C0, C1, MM_DTYPE, OUT_DTYPE)
    if key not in _PROG_CACHE:
        _PROG_CACHE[key] = build_program(C0, C1, MM_DTYPE, OUT_DTYPE)
    nc = _PROG_CACHE[key]
    np_mm = mybir.dt.np(_DT[MM_DTYPE])

    sfc1T = np.ascontiguousarray(sfc1.T)   # [D, S]
    sfc2T = np.ascontiguousarray(sfc2.T)
    sfc3T = np.ascontiguousarray(sfc3.T)   # [S, D]

    # sfc12 per S-half: [KSH, 128, 2*KD*128]
    sfc12_h, sfc3_h = [], []
    for sh in range(PS):
        blk = np.empty((KSH, 128, 2 * KD * 128), np.float32)
        for st in range(KSH):
            s0 = (sh * KSH + st) * 128
            a = sfc1T[:, s0:s0 + 128]    # [D, 128]
            b = sfc2T[:, s0:s0 + 128]
            blk[st, :, : KD * 128] = _pmajor(a, 128)
            blk[st, :, KD * 128:] = _pmajor(b, 128)
        sfc12_h.append(blk.astype(np_mm))
        blk3 = np.empty((KD, 128, KSH * 128), np.float32)
        s0 = sh * SH
        for dt in range(KD):
            # [SH, 128] slice of sfc3T -> partition-major over its s-tiles
            blk3[dt] = _pmajor(
                np.ascontiguousarray(sfc3T[s0:s0 + SH, dt * 128:(dt + 1) * 128]), 128
            )
        sfc3_h.append(blk3.astype(np_mm))

    in_maps = []
    for c in range(NCORES):
        q, sh = c % PT, c // PT
        xqm = _pmajor(
            np.ascontiguousarray(xf[q * TQ:(q + 1) * TQ].T), TQ
        ).astype(np_mm)
        # [128, KD*TQ] -> grid [h*2 + tc, 128, (j%4)*512 + c]: one packed
        # 4KB-per-row transfer per (xq half, token chunk)
        xqm = np.ascontiguousarray(
            xqm.reshape(128, 2, 4, 2, 512).transpose(1, 3, 0, 2, 4).reshape(4, 128, 2048)
        )
        in_map = {
            "xq": xqm,
            "sfc12": sfc12_h[sh],
            "sfc3h": sfc3_h[sh],
        }
        w1_c, w2_c = [], []
        for s, C in ((0, C0), (1, C1)):
            e = slots[s][c]
            rows = toks[e]
            xe = np.zeros((C, D), np.float32)
            xe[: len(rows)] = xf[rows]
            in_map[f"xg{s}"] = _pmajor(np.ascontiguousarray(xe.T), C).astype(np_mm)
            # w1 tiles keyed (ht, j): col block (ht*KD + j) is k-tile j of
            # w1[e].T's h-tile ht
            w1T = np.ascontiguousarray(w1[e].T)   # [D, H]
            w1m = np.empty((128, KH * KD * 128), np.float32)
            for ht in range(KH):
                w1m[:, ht * KD * 128:(ht + 1) * KD * 128] = _pmajor(
                    np.ascontiguousarray(w1T[:, ht * 128:(ht + 1) * 128]), 128
                )
            w1_c.append(w1m)
            # w2 tiles keyed (dt, hj)
            w2T = np.ascontiguousarray(w2[e].T)   # [H, D]
            w2m = np.empty((128, KD * KH * 128), np.float32)
            for dt in range(KD):
                w2m[:, dt * KH * 128:(dt + 1) * KH * 128] = _pmajor(
                    np.ascontiguousarray(w2T[:, dt * 128:(dt + 1) * 128]), 128
                )
            w2_c.append(w2m)
        in_map["w1b"] = np.stack(w1_c).astype(np_mm)
        in_map["w2b"] = np.stack(w2_c).astype(np_mm)
        in_maps.append(in_map)

    trace = TRACE or os.environ.get("BASS_TRACE") == "1"
    res = bass_utils.run_bass_kernel_spmd(
        nc, in_maps, core_ids=list(range(NCORES)), trace=trace
    )
    LAST = res
    results = res.results

    out = np.empty((T, D), np.float32)
    for q in range(PT):
        acc = np.asarray(results[q]["pshout"]).astype(np.float32).reshape(D, TQ)
        acc = acc + np.asarray(results[PT + q]["pshout"]).astype(np.float32).reshape(D, TQ)
        out[q * TQ:(q + 1) * TQ] = acc.T
    for s, C in ((0, C0), (1, C1)):
        for c in range(NCORES):
            e = slots[s][c]
            load = len(toks[e])
            yT = np.asarray(results[c][f"yout{s}"]).astype(np.float32).reshape(D, C)
            out[toks[e]] += wts[e][:, None] * yT[:, :load].T
    return out.reshape(B, L, D)
